# revision 4
# baseline (speedup 1.0000x reference)
"""Trainium2 Bass kernel v4 for the difflogic LogicLayer problem.

Math: out[b,n] = c0 + c1*a + c2*b + c3*a*b  (per-neuron coeffs folded from
softmaxed weights; a = x[b, idx_a[n]], b = x[b, idx_b[n]]).

HW-measured facts driving the design (axon TRN2, per core):
  - dma_gather costs ~4.7 us FIXED on the Pool engine per instruction
    (SWDGE ucode dispatch) + ~1.5 ns/row-descriptor; big gathers win.
  - per-core DMA sustains ~390-420 GB/s; the baseline's 48 MiB/rep f32
    traffic is the wall -> fp16 pipeline (16 MiB gather + 8 MiB store).
  - engine rates (fp16, ns/col): DVE TS 0.51 / TT 0.74, ACT affine 1.10,
    Pool TT 1.92.

Pipeline (CPU-validated max rel err 5.5e-3 vs the 2e-2 gate):
  - host converts x^T to fp16 and folds softmax(weights) -> (c3,c2,c1,c0)
  - supertiles of 1024 neurons: ONE 1024-row gather per operand (4/rep)
  - ACT : u = c1*a + c0          (activation, per-partition scale/bias)
  - DVE : v = c3*a + c2          (tensor_scalar dual, in-place on a)
  - DVE : w = v*b                (tensor_tensor mult, in-place on b)
  - DVE : out = w + u            (chunks 0..kd-1)
  - Pool: out = w + u            (remaining chunks; Pool is idle otherwise)
  - SP  : HWDGE fp16 stores; host upcasts to f32.

Sharding: OUT split 8 ways (2048 neurons/core), x^T replicated.
a/b/u supertile buffers all double-buffered: 6 x 32 KiB/partition = 192 KiB
of the ~208 KiB usable SBUF.
"""

import sys

import numpy as np

sys.path.insert(0, "/opt/trn_rl_repo")

B = 2048
IN_DIM = 16384
OUT_DIM = 16384
N_CORES = 8

OUT_PER_CORE = OUT_DIM // N_CORES  # 2048
PART = 128
CHUNK = 8                          # 128-neuron chunks per supertile
POOL_ADD_CHUNKS = 3                # final-add chunks handled by Pool (of 8)
TILE_IDX = PART * CHUNK            # 1024 neurons per gather/supertile
NT = OUT_PER_CORE // TILE_IDX      # 2 supertiles per core
IDX_COLS = TILE_IDX // 16          # 64 int16 index columns per supertile

LAST_EXEC_TIME_NS = None
LAST_RESULTS = None

_GATE_BASIS = np.array(
    [
        # const  a    b    ab
        [0, 0, 0, 0],    # FALSE
        [0, 0, 0, 1],    # a AND b
        [0, 1, 0, -1],   # a AND NOT b
        [0, 1, 0, 0],    # a
        [0, 0, 1, -1],   # NOT a AND b
        [0, 0, 1, 0],    # b
        [0, 1, 1, -2],   # XOR
        [0, 1, 1, -1],   # OR
        [1, -1, -1, 1],  # NOR
        [1, -1, -1, 2],  # XNOR
        [1, 0, -1, 0],   # NOT b
        [1, 0, -1, 1],   # a OR NOT b
        [1, -1, 0, 0],   # NOT a
        [1, -1, 0, 1],   # NOT a OR b
        [1, 0, 0, -1],   # NAND
        [1, 0, 0, 0],    # TRUE
    ],
    dtype=np.float64,
)


def _coeffs_from_weights(weights: np.ndarray) -> np.ndarray:
    """softmax(weights) -> [OUT, 4] scalars ordered (c3, c2, c1, c0)."""
    w = weights.astype(np.float64)
    w = w - w.max(axis=-1, keepdims=True)
    p = np.exp(w)
    p /= p.sum(axis=-1, keepdims=True)
    c = p @ _GATE_BASIS  # columns: c0, c1, c2, c3
    return np.stack([c[:, 3], c[:, 2], c[:, 1], c[:, 0]], axis=-1)


_NC_CACHE = {}


def _build_bass(n_rows, elem, nt, chunk=CHUNK, part=PART, n_rep=1,
                pool_chunks=POOL_ADD_CHUNKS):
    """One-core SPMD program; all cores run the same code on different inputs."""
    import concourse.bacc as bacc
    import concourse.mybir as mybir
    from concourse.library_config import mlp

    tile_idx = part * chunk
    idx_cols = tile_idx // 16
    ntot = nt * n_rep
    kd = chunk - pool_chunks       # final-add chunks on DVE

    f32 = mybir.dt.float32
    f16 = mybir.dt.float16
    i16 = mybir.dt.int16
    mult = mybir.AluOpType.mult
    add = mybir.AluOpType.add
    Ident = mybir.ActivationFunctionType.Identity

    nc = bacc.Bacc("TRN2")
    xt = nc.dram_tensor("xt", [n_rows, elem], f16, kind="ExternalInput")
    idxa = nc.dram_tensor("idxa", [part, nt * idx_cols], i16, kind="ExternalInput")
    idxb = nc.dram_tensor("idxb", [part, nt * idx_cols], i16, kind="ExternalInput")
    coef = nc.dram_tensor("coef", [part, nt * chunk * 4], f32, kind="ExternalInput")
    out = nc.dram_tensor("out", [nt, part, chunk, elem], f16, kind="ExternalOutput")

    from contextlib import ExitStack
    with ExitStack() as _stack:
        ec = _stack.enter_context
        idxa_s = ec(nc.sbuf_tensor("idxa_s", [part, nt * idx_cols], i16))
        idxb_s = ec(nc.sbuf_tensor("idxb_s", [part, nt * idx_cols], i16))
        coef_s = ec(nc.sbuf_tensor("coef_s", [part, nt * chunk * 4], f32))
        a0 = ec(nc.sbuf_tensor("a0", [part, chunk, elem], f16))
        a1 = ec(nc.sbuf_tensor("a1", [part, chunk, elem], f16))
        b0 = ec(nc.sbuf_tensor("b0", [part, chunk, elem], f16))
        b1 = ec(nc.sbuf_tensor("b1", [part, chunk, elem], f16))
        u0 = ec(nc.sbuf_tensor("u0", [part, chunk, elem], f16))
        u1 = ec(nc.sbuf_tensor("u1", [part, chunk, elem], f16))
        ld = ec(nc.semaphore("ld"))
        ga0 = ec(nc.semaphore("ga0"))
        ga1 = ec(nc.semaphore("ga1"))
        gb0 = ec(nc.semaphore("gb0"))
        gb1 = ec(nc.semaphore("gb1"))
        act_sem = ec(nc.semaphore("act"))
        dve_sem = ec(nc.semaphore("dve"))
        pa_sem = ec(nc.semaphore("pa"))
        st0 = ec(nc.semaphore("st0"))
        st1 = ec(nc.semaphore("st1"))
        block = ec(nc.Block())
        a_bufs, b_bufs, u_bufs = [a0, a1], [b0, b1], [u0, u1]
        ga_sems, gb_sems, st_sems = [ga0, ga1], [gb0, gb1], [st0, st1]

        # per-supertile semaphore increments
        DVE_INC = chunk + 1 + kd   # 8 TS + 1 TT + kd adds
        PA_INC = pool_chunks

        def sc(t, c, k):
            # scalar AP for supertile t, chunk c, coef k (0:c3, 1:c2, 2:c1, 3:c0)
            col = (t * chunk + c) * 4 + k
            return coef_s[:, col:col + 1]

        @block.sync
        def _(sync):
            sync.dma_start(idxa_s[:, :], idxa[:, :]).then_inc(ld, 16)
            sync.dma_start(idxb_s[:, :], idxb[:, :]).then_inc(ld, 16)
            sync.dma_start(coef_s[:, :], coef[:, :]).then_inc(ld, 16)
            for S in range(ntot):
                t, j = S % nt, S % 2
                sync.wait_ge(dve_sem, DVE_INC * S + DVE_INC)
                sync.wait_ge(pa_sem, PA_INC * S + PA_INC)
                sync.dma_start(out[t, :, :, :], b_bufs[j][:, :, :]).then_inc(
                    st_sems[j], 16)
            sync.wait_ge(st0, 16 * ((ntot + 1) // 2))
            sync.wait_ge(st1, 16 * (ntot // 2))

        @block.gpsimd
        def _(gpsimd):
            gpsimd.load_library(mlp)
            gpsimd.wait_ge(ld, 48)

            def issue_gathers(S):
                t, j = S % nt, S % 2
                if S >= 2:
                    # a[j] free once TT(S-2) (inc #chunk+1 of its tile) is done
                    gpsimd.wait_ge(dve_sem, DVE_INC * (S - 2) + chunk + 1)
                gpsimd.dma_gather(
                    a_bufs[j][:, :, :], xt[:, :],
                    idxa_s[:, t * idx_cols:(t + 1) * idx_cols],
                    tile_idx, tile_idx, elem,
                ).then_inc(ga_sems[j], 16)
                if S >= 2:
                    gpsimd.wait_ge(st_sems[j], 16 * (S // 2))  # b[j] free
                gpsimd.dma_gather(
                    b_bufs[j][:, :, :], xt[:, :],
                    idxb_s[:, t * idx_cols:(t + 1) * idx_cols],
                    tile_idx, tile_idx, elem,
                ).then_inc(gb_sems[j], 16)

            def issue_adds(S):
                t, j = S % nt, S % 2
                gpsimd.wait_ge(dve_sem, DVE_INC * S + chunk + 1)  # w ready
                gpsimd.wait_ge(act_sem, chunk * S + chunk)        # u ready
                for c in range(kd, chunk):
                    gpsimd.tensor_tensor(
                        b_bufs[j][:, c, :], b_bufs[j][:, c, :],
                        u_bufs[j][:, c, :], add,
                    ).then_inc(pa_sem, 1)

            issue_gathers(0)
            for S in range(1, ntot):
                issue_gathers(S)
                issue_adds(S - 1)
            issue_adds(ntot - 1)

        @block.scalar
        def _(scalar):
            for S in range(ntot):
                t, j = S % nt, S % 2
                scalar.wait_ge(ga_sems[j], 16 * (S // 2 + 1))
                if S >= 2:  # u[j] free once both add groups of S-2 are done
                    scalar.wait_ge(dve_sem, DVE_INC * (S - 2) + DVE_INC)
                    scalar.wait_ge(pa_sem, PA_INC * (S - 2) + PA_INC)
                for c in range(chunk):
                    scalar.activation(
                        u_bufs[j][:, c, :], a_bufs[j][:, c, :], Ident,
                        bias=sc(t, c, 3), scale=sc(t, c, 2),
                    ).then_inc(act_sem, 1)

        @block.vector
        def _(vector):
            for S in range(ntot):
                t, j = S % nt, S % 2
                for c in range(chunk):
                    vector.wait_ge(act_sem, chunk * S + c + 1)  # ACT read a[c]
                    vector.tensor_scalar(
                        a_bufs[j][:, c, :], a_bufs[j][:, c, :],
                        sc(t, c, 0), sc(t, c, 1), mult, add,
                    ).then_inc(dve_sem, 1)
                vector.wait_ge(gb_sems[j], 16 * (S // 2 + 1))
                vector.wait_ge(dve_sem, DVE_INC * S + chunk)  # own TS visible
                vector.tensor_tensor(
                    b_bufs[j][:, :, :], a_bufs[j][:, :, :], b_bufs[j][:, :, :],
                    mult,
                ).then_inc(dve_sem, 1)
                for c in range(kd):
                    vector.wait_ge(act_sem, chunk * S + chunk)
                    vector.wait_ge(dve_sem, DVE_INC * S + chunk + 1)
                    vector.tensor_tensor(
                        b_bufs[j][:, c, :], b_bufs[j][:, c, :],
                        u_bufs[j][:, c, :], add,
                    ).then_inc(dve_sem, 1)

    nc.compile()
    return nc


def _pack_idx(idx: np.ndarray, nt: int, tile_idx: int) -> np.ndarray:
    """Pack per-core indices into the dma_gather SBUF layout:
    tile t's index i lives at [i % 16, t*idx_cols + i//16] (int16)."""
    idx_cols = tile_idx // 16
    v = idx.astype(np.int16).reshape(nt, idx_cols, 16)  # [t, s, p], i = s*16+p
    block16 = v.transpose(2, 0, 1).reshape(16, nt * idx_cols)
    return np.ascontiguousarray(np.tile(block16, (PART // 16, 1)))


def _pack_coef(cc: np.ndarray, nt: int, chunk: int) -> np.ndarray:
    """cc: [OUT_PER_CORE, 4] -> [128, nt*chunk*4]; neuron t*chunk*128 + c*128+p
    lands at [p, (t*chunk + c)*4 + k]."""
    return np.ascontiguousarray(
        cc.reshape(nt, chunk, PART, 4).transpose(2, 0, 1, 3)
        .reshape(PART, nt * chunk * 4).astype(np.float32)
    )


def prepare(inputs, n_rep=1):
    """Build (nc, in_maps) for the SPMD run."""
    x = np.asarray(inputs["x"], dtype=np.float32)
    weights = np.asarray(inputs["weights"], dtype=np.float32)
    idx_a = np.asarray(inputs["idx_a"])
    idx_b = np.asarray(inputs["idx_b"])

    cc = _coeffs_from_weights(weights)                    # [OUT, 4]
    xt16 = np.ascontiguousarray(x.T.astype(np.float16))   # [IN, B] fp16 rows

    key = ("v4", IN_DIM, B, NT, n_rep)
    if key not in _NC_CACHE:
        _NC_CACHE[key] = _build_bass(IN_DIM, B, NT, n_rep=n_rep)
    nc = _NC_CACHE[key]

    in_maps = []
    for c in range(N_CORES):
        n0 = c * OUT_PER_CORE
        n1 = n0 + OUT_PER_CORE
        in_maps.append({
            "xt": xt16,
            "idxa": _pack_idx(idx_a[n0:n1], NT, TILE_IDX),
            "idxb": _pack_idx(idx_b[n0:n1], NT, TILE_IDX),
            "coef": _pack_coef(cc[n0:n1], NT, CHUNK),
        })
    return nc, in_maps


def assemble(results):
    """results: per-core dicts with 'out' [NT, 128, CHUNK, B] f16 ->
    full [B, OUT] f32."""
    outs = []
    for c in range(N_CORES):
        o = results[c]["out"]
        outs.append(o.transpose(0, 2, 1, 3).reshape(OUT_PER_CORE, B))
    full = np.concatenate(outs, axis=0)  # [OUT, B] f16
    return np.ascontiguousarray(full.T).astype(np.float32)


def kernel(x, weights, idx_a, idx_b):
    global LAST_EXEC_TIME_NS, LAST_RESULTS
    from concourse.bass_utils import run_bass_kernel_spmd

    nc, in_maps = prepare(
        {"x": x, "weights": weights, "idx_a": idx_a, "idx_b": idx_b}
    )
    res = run_bass_kernel_spmd(nc, in_maps, list(range(N_CORES)))
    LAST_EXEC_TIME_NS = res.exec_time_ns
    LAST_RESULTS = res
    return assemble(res.results)
